# revision 21
# baseline (speedup 1.0000x reference)
"""Trainium2 Bass kernel for nn_HashingModel (retrieval_knn).

Sharding: data-parallel over batch B across 8 cores (256 rows each).
Cross-batch MHA handled by AllGather of the per-shard K/V projections
(split into separate kp and vp collectives so phase C can start on kp).
All heavy matmuls in bf16; similarity in fp32r (FP22 on the PE - 2x the
fp32 rate, zero argmax flips verified offline against fp32).
Activations flow transposed ([feature, batch] layouts) so weights can be
used pre-transposed (host-side layout prep) without on-device transposes.
Softmax skips max-subtraction (scores are tiny: weights scaled 0.02); the
denominator comes free from a ones-column appended to V.

Orchestration: three DMA lanes to avoid head-of-line blocking - sync
carries the latency-critical chain (x, prompts, idx, kp/vp shards, kpT),
scalar carries phase-A/B weight prefetch + h1x streams, gpsimd carries
vaug + phase-D weight streams (safe: its phase-C work precedes them in
queue order). PE program order: sim(i), sim(t), kvproj(i), qp(i), qp(t),
h1x(img), kvproj(t), mha(i), mha(t), h1x(txt), MLPs. The h1x precompute
(W1 @ x-half) fills the PE while the AllGathers are in flight.
Final Wc stage keeps BIT on partitions (weights stationary) and
transposes the [BIT, batch] result back via the PE with an identity.

Self-contained: hardcoded shapes, no file reads.
"""
import sys
import numpy as np

sys.path.insert(0, '/opt/trn_rl_repo')

import ml_dtypes
from concourse import bass, bacc, tile, mybir
from concourse.bass_utils import run_bass_kernel_spmd

dt = mybir.dt
BF16 = ml_dtypes.bfloat16
AF = mybir.ActivationFunctionType
F32R = dt.float32r

FULL = dict(NC=8, B=2048, E=512, P=4096, H=8, HD=64, HID=4096, BIT=64)


def _cfg(NC, B, E, P, H, HD, HID, BIT):
    c = dict(NC=NC, B=B, E=E, P=P, H=H, HD=HD, HID=HID, BIT=BIT)
    c['BS'] = B // NC          # batch shard per core
    c['E2'] = 2 * E            # MLP input dim
    c['EPAD'] = ((E + 1 + 127) // 128) * 128  # padded prompt row (ones col at E)
    c['KT_E'] = E // 128       # k-tiles over E
    c['KT_E2'] = 2 * E // 128
    c['NT_HID'] = HID // 128
    c['LT'] = c['BS'] // 128   # l-tiles per shard
    c['ST'] = B // 128         # s-tiles over full batch
    c['PC'] = P // 512         # prompt chunks for sim
    c['B2'] = 2 * c['BS']      # MLP free dim (fi|ft)
    return c


def build_nc(cfg, n_cores):
    C = cfg
    NC = n_cores
    E, P, H, HD, HID, BIT = C['E'], C['P'], C['H'], C['HD'], C['HID'], C['BIT']
    BS, E2, EPAD = C['BS'], C['E2'], C['EPAD']
    KT_E, KT_E2, NT_HID, LT, ST, PC, B2 = (C['KT_E'], C['KT_E2'], C['NT_HID'],
                                           C['LT'], C['ST'], C['PC'], C['B2'])
    HPT = 128 // HD            # heads per 128-partition tile (2)
    NHT = E // 128             # eo tiles (4)
    SEG = HD + 1               # vaug segment width (65)
    S = C['ST'] * 128          # full batch (attention keys)

    nc = bacc.Bacc("TRN2", target_bir_lowering=False, debug=False,
                   num_devices=NC)

    mods = ['i', 't']
    inp = {}

    def din(name, shape, d):
        inp[name] = nc.dram_tensor(name, shape, d, kind="ExternalInput")

    for m in mods:
        din(f'xT_{m}', [E, BS], dt.float32)
        for w in ['wqT', 'wkT', 'wvT', 'woT']:
            din(f'{w}_{m}', [E, E], dt.bfloat16)
        din(f'bq_{m}', [E], dt.float32)
        din(f'bo_{m}', [E], dt.float32)
        din(f'bk_{m}', [1, E], dt.bfloat16)
        din(f'bv_{m}', [1, E], dt.bfloat16)
    din('promptsT', [E, P], dt.float32)
    din('prompts_pad', [P, EPAD], dt.bfloat16)
    din('ident64', [BIT, BIT], dt.float32)
    for M in ['img', 'txt']:
        din(f'w1T_{M}', [NT_HID, 128, KT_E2, 128], dt.bfloat16)
        din(f'w2T_{M}', [NT_HID, 128, NT_HID, 128], dt.bfloat16)
        din(f'wcT_{M}', [128, NT_HID, BIT], dt.bfloat16)
        din(f'b1_{M}', [HID], dt.float32)
        din(f'b2_{M}', [HID], dt.float32)
        din(f'bcT_{M}', [BIT], dt.float32)

    outs = {}
    for name in ['image_hash', 'text_hash', 'distill_i', 'distill_t']:
        outs[name] = nc.dram_tensor(name, [BS, BIT], dt.float32,
                                    kind="ExternalOutput")

    idx_scr = {m: nc.dram_tensor(f'idx_scr_{m}', [BS], dt.uint32) for m in mods}
    h1x_dram = {M: nc.dram_tensor(f'h1x_{M}', [NT_HID, 128, B2], dt.bfloat16)
                for M in ['img', 'txt']}
    kv_len = 2 * E * BS
    kv_in = {m: nc.dram_tensor(f'kv_in_{m}', [kv_len], dt.bfloat16) for m in mods}
    kv_out = {m: nc.dram_tensor(f'kv_out_{m}', [NC * kv_len], dt.bfloat16,
                                addr_space="Shared") for m in mods}

    with tile.TileContext(nc) as tc:
      with tc.tile_pool(name="persist", bufs=1) as pp:
        xTbf = {m: pp.tile([128, KT_E, BS], dt.bfloat16, tag=f'xTbf{m}', name=f'xTbf{m}')
                for m in mods}
        inT = pp.tile([128, KT_E2, B2], dt.bfloat16, tag='inT')
        rmT = {m: pp.tile([128, EPAD // 128, BS], dt.bfloat16, tag=f'rmT{m}', name=f'rmT{m}')
               for m in mods}
        wq = {m: pp.tile([128, KT_E, E], dt.bfloat16, tag=f'wq{m}', name=f'wq{m}')
              for m in mods}
        wo = {m: pp.tile([128, KT_E, E], dt.bfloat16, tag=f'wo{m}', name=f'wo{m}')
              for m in mods}
        bq = {m: pp.tile([128, NHT], dt.float32, tag=f'bq{m}', name=f'bq{m}')
              for m in mods}
        bo = {m: pp.tile([128, NHT], dt.float32, tag=f'bo{m}', name=f'bo{m}')
              for m in mods}
        qpT = {m: pp.tile([128, NHT, BS], dt.bfloat16, tag=f'qpT{m}', name=f'qpT{m}')
               for m in mods}

        with (
            tc.tile_pool(name="phB", bufs=1) as bp_,
            tc.tile_pool(name="smB", bufs=2) as spB,
            tc.tile_pool(name="psB1", bufs=2, space="PSUM") as psB1,
        ):
            wkv = {}
            for m in mods:
                wk = bp_.tile([128, KT_E, E], dt.bfloat16, tag=f'wk{m}', name=f'wk{m}')
                wv = bp_.tile([128, KT_E, E], dt.bfloat16, tag=f'wv{m}', name=f'wv{m}')
                bk = bp_.tile([1, E], dt.bfloat16, tag=f'bk{m}', name=f'bk{m}')
                bv = bp_.tile([1, E], dt.bfloat16, tag=f'bv{m}', name=f'bv{m}')
                wkv[m] = (wk, wv, bk, bv)

            # ---- phase A: sims + argmax + gather (big tiles scoped so the
            # pool closes before phase C's K/V tiles open) ----
            with (
                tc.tile_pool(name="phSim", bufs=1) as ap_,
                tc.tile_pool(name="simbuf", bufs=3) as simp,
                tc.tile_pool(name="smA", bufs=2) as sp,
                tc.tile_pool(name="psA", bufs=3, space="PSUM") as psA,
            ):
                # sync lane: x then prompt chunks (sim critical path)
                xT32 = {m: ap_.tile([128, KT_E, BS], F32R, tag=f'xT32{m}', name=f'xT32{m}')
                        for m in mods}
                for m in mods:
                    xsrc = inp[f'xT_{m}'].ap().bitcast(F32R).rearrange(
                        "(g k p) b -> g p k b", g=2, p=128)
                    nc.sync.dma_start(xT32[m][:, 0:KT_E // 2, :], xsrc[0])
                    nc.scalar.dma_start(xT32[m][:, KT_E // 2:KT_E, :], xsrc[1])
                prc = []
                prsrc = inp['promptsT'].ap().bitcast(F32R).rearrange("(k p) n -> k p n", p=128)
                lanes = [nc.sync, nc.scalar, nc.gpsimd]
                prsrc2 = inp['promptsT'].ap().bitcast(F32R).rearrange(
                    "(g k p) n -> g p k n", g=KT_E // 2, p=128)
                for pc in range(PC):
                    t = ap_.tile([128, KT_E, 512], F32R, tag=f'prc{pc}',
                                 name=f'prc{pc}')
                    if pc == 0:
                        for k in range(KT_E):
                            nc.sync.dma_start(
                                t[:, k, :], prsrc[k][:, 0:512])
                    else:
                        for g in range(KT_E // 2):
                            lane = lanes[(pc * (KT_E // 2) + g) % 3]
                            lane.dma_start(
                                t[:, 2 * g:2 * (g + 1), :],
                                prsrc2[g][:, :, pc * 512:(pc + 1) * 512])
                    prc.append(t)
                # scalar lane: MHA weights + biases (needed from ~70us)
                for m in mods:
                    wk, wv, bk, bv = wkv[m]
                    nc.scalar.dma_start(
                        wk[:], inp[f'wkT_{m}'].ap().rearrange("(k p) n -> p k n", p=128))
                    nc.scalar.dma_start(
                        wv[:], inp[f'wvT_{m}'].ap().rearrange("(k p) n -> p k n", p=128))
                    nc.scalar.dma_start(bk[:], inp[f'bk_{m}'].ap())
                    nc.scalar.dma_start(bv[:], inp[f'bv_{m}'].ap())
                    nc.scalar.dma_start(
                        wq[m][:], inp[f'wqT_{m}'].ap().rearrange("(k p) n -> p k n", p=128))
                    nc.scalar.dma_start(
                        wo[m][:], inp[f'woT_{m}'].ap().rearrange("(k p) n -> p k n", p=128))
                    nc.scalar.dma_start(
                        bq[m][:], inp[f'bq_{m}'].ap().rearrange("(t p) -> p t", p=128))
                    nc.scalar.dma_start(
                        bo[m][:], inp[f'bo_{m}'].ap().rearrange("(t p) -> p t", p=128))
                for mi, m in enumerate(mods):
                    nc.vector.tensor_copy(xTbf[m][:], xT32[m][:])
                    nc.vector.tensor_copy(inT[:, 0:KT_E, mi * BS:(mi + 1) * BS],
                                          xTbf[m][:])

                for m in mods:
                    FRL = 128 // 16
                    for lt in range(LT):
                        sim = simp.tile([128, P], dt.float32, tag='sim')
                        xs = xT32[m][:, :, lt * 128:(lt + 1) * 128]
                        for pc in range(PC):
                            ps = psA.tile([128, 512], dt.float32, tag='ps_sim')
                            for k in range(KT_E):
                                nc.tensor.matmul(
                                    ps[:], xs[:, k, :], prc[pc][:, k, :],
                                    start=(k == 0), stop=(k == KT_E - 1))
                            nc.vector.tensor_copy(
                                sim[:, pc * 512:(pc + 1) * 512], ps[:])
                        m8 = sp.tile([128, 8], dt.float32, tag='m8')
                        i8 = sp.tile([128, 8], dt.uint32, tag=f'i8_{lt}',
                                     name=f'i8_{lt}')
                        nc.vector.max(m8[:], sim[:])
                        nc.vector.max_index(i8[:], m8[:], sim[:])
                        nc.sync.dma_start(
                            idx_scr[m].ap()[lt * 128:(lt + 1) * 128], i8[:, 0:1])
                        # per-lt gather: start fetching prompts for this half
                        # of the shard while the other half's sims still run
                        i32 = sp.tile([128, FRL], dt.uint32, tag=f'i32g{lt}',
                                      name=f'i32g{lt}')
                        for a in range(2):
                            nc.sync.dma_start(
                                i32[16 * a:16 * (a + 1), :],
                                idx_scr[m].ap()[lt * 128:(lt + 1) * 128]
                                .rearrange("(f p) -> p f", p=16))
                        for b in range(1, 4):
                            nc.vector.tensor_copy(i32[32 * b:32 * (b + 1), :],
                                                  i32[0:32, :])
                        ix16 = sp.tile([128, FRL], dt.int16, tag=f'i16g{lt}',
                                       name=f'i16g{lt}')
                        nc.vector.tensor_copy(ix16[:], i32[:])
                        rmg = sp.tile([128, EPAD // 128, 128], dt.bfloat16,
                                      tag=f'rmg{lt}', name=f'rmg{lt}')
                        nc.gpsimd.dma_gather(
                            rmg[:], inp['prompts_pad'].ap(), ix16[:],
                            num_idxs=128, num_idxs_reg=128, elem_size=EPAD,
                            transpose=True)
                        nc.vector.tensor_copy(
                            rmT[m][:, :, lt * 128:(lt + 1) * 128], rmg[:])

            # ---- phases B + C ----
            with (
                tc.tile_pool(name="phC", bufs=1) as cp_,
                tc.tile_pool(name="expp", bufs=3) as ep,
                tc.tile_pool(name="smC", bufs=1) as spC,
                tc.tile_pool(name="psO", bufs=1, space="PSUM") as psO,
                tc.tile_pool(name="psS", bufs=2, space="PSUM") as psS,
            ):
                def kvproj(m):
                    wk, wv, bk, bv = wkv[m]
                    # kpT shard -> kp_in laid out [KT_E*128, BS]
                    kpsb = spB.tile([128, NHT, BS], dt.bfloat16, tag='kpsb')
                    for eo in range(NHT):
                        psb = psB1.tile([128, 512], dt.float32, tag='ps_b',
                                        name='ps_bk')
                        ps = psb[:, 0:BS]
                        for k in range(KT_E):
                            nc.tensor.matmul(
                                ps, wk[:, k, eo * 128:(eo + 1) * 128],
                                rmT[m][:, k, :], start=(k == 0), stop=False)
                        nc.tensor.matmul(
                            ps, bk[0:1, eo * 128:(eo + 1) * 128],
                            rmT[m][0:1, KT_E, :], start=False, stop=True)
                        nc.vector.tensor_copy(kpsb[:, eo, :], ps)
                    nc.sync.dma_start(
                        kv_in[m].ap()[0:E * BS]
                        .rearrange("(q p b) -> p q b", p=128, b=BS),
                        kpsb[:])
                    # vp shard -> kv_in[E*BS:] laid out [LT*128, E]
                    vpsb = spB.tile([128, LT, E], dt.bfloat16, tag='vpsb')
                    for st in range(LT):
                        ps = psB1.tile([128, E], dt.float32, tag='ps_b',
                                       name='ps_bv')
                        for k in range(KT_E):
                            nc.tensor.matmul(
                                ps[:], rmT[m][:, k, st * 128:(st + 1) * 128],
                                wv[:, k, :], start=(k == 0), stop=False)
                        nc.tensor.matmul(
                            ps[:], rmT[m][0:1, KT_E, st * 128:(st + 1) * 128],
                            bv[:], start=False, stop=True)
                        nc.vector.tensor_copy(vpsb[:, st, :], ps[:])
                    nc.sync.dma_start(
                        kv_in[m].ap()[E * BS:]
                        .rearrange("(f p b) -> p f b", p=128, b=E),
                        vpsb[:])
                    nc.gpsimd.collective_compute(
                        "AllGather", mybir.AluOpType.bypass,
                        replica_groups=[list(range(NC))],
                        ins=[kv_in[m][:]], outs=[kv_out[m][:]])

                kpT = {}
                vaug = {}

                def load_kpT(m):
                    kpT[m] = cp_.tile([128, NHT, S], dt.bfloat16,
                                      tag=f'kpT{m}', name=f'kpT{m}')
                    ksrc = kv_out[m].ap().rearrange(
                        "(r q p b) -> q p r b", r=NC, q=2 * E // 128, p=128)
                    for eo in range(NHT):
                        lane = nc.sync if eo % 2 == 0 else nc.scalar
                        lane.dma_start(
                            kpT[m][:, eo, :].rearrange("p (r b) -> p r b", r=NC),
                            ksrc[eo])

                def load_vaug(m):
                    vaug[m] = cp_.tile([128, ST, H * SEG], dt.bfloat16,
                                       tag=f'vaug{m}', name=f'vaug{m}')
                    nc.vector.memset(
                        vaug[m][:].rearrange("p s (h d) -> p s h d", h=H)
                        [:, :, :, HD:HD + 1], 1.0)
                    vsrc = kv_out[m].ap().rearrange("(r x) -> r x", r=NC)
                    for st in range(ST):
                        r, hf = st // LT, st % LT
                        blk = vsrc[r][E * BS + hf * 128 * E:
                                      E * BS + (hf + 1) * 128 * E] \
                            .rearrange("(p h d) -> p h d", p=128, h=H)
                        nc.gpsimd.dma_start(
                            vaug[m][:, st, :].rearrange("p (h s) -> p h s", h=H)
                            [:, :, 0:HD], blk)

                def qproj(m):
                    for eo in range(NHT):
                        psb = psB1.tile([128, 512], dt.float32, tag='ps_b',
                                        name='ps_q')
                        ps = psb[:, 0:BS]
                        for k in range(KT_E):
                            nc.tensor.matmul(
                                ps, wq[m][:, k, eo * 128:(eo + 1) * 128],
                                xTbf[m][:, k, :], start=(k == 0),
                                stop=(k == KT_E - 1))
                        nc.vector.tensor_scalar_add(qpT[m][:, eo, :], ps,
                                                    bq[m][:, eo:eo + 1])

                def h1x(M):
                    # h1x = W1[:, x-half] @ [xT_i | xT_t] + b1 -> DRAM
                    # streams over the scalar DMA lane, 4-ht groups
                    b1x = spB.tile([128, NT_HID], dt.float32, tag='b1x')
                    nc.scalar.dma_start(
                        b1x[:], inp[f'b1_{M}'].ap().rearrange("(t p) -> p t", p=128))
                    for ht0 in range(0, NT_HID, 4):
                        wblk4 = spB.tile([128, 4, KT_E, 128], dt.bfloat16,
                                         tag='w1xblk')
                        nc.scalar.dma_start(
                            wblk4[:],
                            inp[f'w1T_{M}'].ap()[ht0:ht0 + 4][:, :, 0:KT_E]
                            .rearrange("h p k c -> p h k c"))
                        hx4 = spB.tile([128, 4, B2], dt.bfloat16, tag='h1x_sb')
                        for j in range(4):
                            ps = psB1.tile([128, B2], dt.float32, tag='ps_b',
                                           name='ps_h1x')
                            for k in range(KT_E):
                                nc.tensor.matmul(ps[:], wblk4[:, j, k, :],
                                                 inT[:, k, :],
                                                 start=(k == 0),
                                                 stop=(k == KT_E - 1))
                            nc.vector.tensor_scalar_add(
                                hx4[:, j, :], ps[:], b1x[:, ht0 + j:ht0 + j + 1])
                        nc.scalar.dma_start(
                            h1x_dram[M].ap()[ht0:ht0 + 4]
                            .rearrange("h p b -> p h b"), hx4[:])

                def mha(m, mi):
                    pso = [psO.tile([SEG, HPT * BS], dt.float32, tag=f'pso{g}',
                                    name=f'pso{g}') for g in range(H // HPT)]
                    for st2 in range(0, ST, 2):
                        ex = ep.tile([128, H, 2 * BS], dt.bfloat16, tag='expT')
                        for g in range(H // HPT):
                            for hh in range(HPT):
                                h = g * HPT + hh
                                hb = hh * HD
                                # two s-tiles share one psum bank: same PE
                                # row-group -> sequential drains, one exp op
                                pss = psS.tile([128, 2 * BS], dt.float32,
                                               tag='ps_s')
                                for sj in range(2):
                                    st = st2 + sj
                                    nc.tensor.matmul(
                                        pss[:, sj * BS:(sj + 1) * BS],
                                        kpT[m][hb:hb + HD, g,
                                               st * 128:(st + 1) * 128],
                                        qpT[m][hb:hb + HD, g, :],
                                        start=True, stop=True,
                                        skip_group_check=True)
                                nc.scalar.activation(
                                    ex[:, h, :], pss[:], AF.Exp,
                                    bias=0.0, scale=float(1.0 / np.sqrt(HD)))
                                for sj in range(2):
                                    st = st2 + sj
                                    nc.tensor.matmul(
                                        pso[g][:, hh * BS:(hh + 1) * BS],
                                        vaug[m][:, st, h * SEG:(h + 1) * SEG],
                                        ex[:, h, sj * BS:(sj + 1) * BS],
                                        start=(st == 0), stop=(st == ST - 1),
                                        skip_group_check=True)
                    zr = spC.tile([1, H * BS], dt.float32, tag='zr')
                    for h in range(H):
                        nc.vector.reciprocal(
                            zr[0:1, h * BS:(h + 1) * BS],
                            pso[h // HPT][HD:HD + 1, (h % HPT) * BS:(h % HPT + 1) * BS])
                    zb = spC.tile([HD, H * BS], dt.float32, tag='zb')
                    nc.gpsimd.partition_broadcast(zb[:], zr[:])
                    aoT = cp_.tile([128, NHT, BS], dt.bfloat16, tag='aoT')
                    for h in range(H):
                        nc.vector.tensor_tensor(
                            out=aoT[(h % HPT) * HD:(h % HPT + 1) * HD, h // HPT, :],
                            in0=pso[h // HPT][0:HD, (h % HPT) * BS:(h % HPT + 1) * BS],
                            in1=zb[:, h * BS:(h + 1) * BS],
                            op=mybir.AluOpType.mult)

                    # enhT -> inT rows E..2E-1; x -> rows 0..E-1
                    for eo in range(NHT):
                        psb = psB1.tile([128, 512], dt.float32, tag='ps_b',
                                        name='ps_e')
                        ps = psb[:, 0:BS]
                        for k in range(KT_E):
                            nc.tensor.matmul(
                                ps, wo[m][:, k, eo * 128:(eo + 1) * 128],
                                aoT[:, k, :], start=(k == 0), stop=(k == KT_E - 1))
                        nc.vector.tensor_scalar_add(
                            inT[:, KT_E + eo, mi * BS:(mi + 1) * BS], ps,
                            bo[m][:, eo:eo + 1])

                kvproj('i')
                qproj('i')
                qproj('t')
                h1x('img')
                kvproj('t')
                h1x('txt')
                load_kpT('i')
                load_vaug('i')
                load_kpT('t')
                load_vaug('t')
                mha('i', 0)
                mha('t', 1)

        # ======== Phase D: the four MLPs (two weight passes) ========
        with (
            tc.tile_pool(name="phD", bufs=1) as dp_,
            tc.tile_pool(name="w1s", bufs=2) as wp,
            tc.tile_pool(name="w2s", bufs=3) as w2p,
            tc.tile_pool(name="smD", bufs=2) as spD,
            tc.tile_pool(name="psD", bufs=4, space="PSUM") as psD,
            tc.tile_pool(name="psW", bufs=1, space="PSUM") as psW,
            tc.tile_pool(name="psT", bufs=2, space="PSUM") as psT,
        ):
            h1T = dp_.tile([128, NT_HID, B2], dt.bfloat16, tag='h1T')
            h2T = dp_.tile([128, NT_HID, B2], dt.bfloat16, tag='h2T')
            identT = dp_.tile([BIT, BIT], dt.float32, tag='ident')
            nc.sync.dma_start(identT[:], inp['ident64'].ap())
            out_map = {'img': ['image_hash', 'distill_i'],
                       'txt': ['distill_t', 'text_hash']}
            for M in ['img', 'txt']:
                b2 = spD.tile([128, NT_HID], dt.float32, tag='b2')
                bcT = spD.tile([BIT, 1], dt.float32, tag='bcT')
                nc.sync.dma_start(
                    b2[:], inp[f'b2_{M}'].ap().rearrange("(t p) -> p t", p=128))
                nc.sync.dma_start(
                    bcT[:], inp[f'bcT_{M}'].ap().rearrange("(p o) -> p o", p=BIT))

                for ht0 in range(0, NT_HID, 4):
                    wblk4 = wp.tile([128, 4, KT_E, 128], dt.bfloat16,
                                    tag='w1blk')
                    lane = nc.gpsimd if (ht0 // 4) % 2 == 0 else nc.scalar
                    lane.dma_start(
                        wblk4[:],
                        inp[f'w1T_{M}'].ap()[ht0:ht0 + 4][:, :, KT_E:KT_E2]
                        .rearrange("h p k c -> p h k c"))
                    hx4 = wp.tile([128, 4, B2], dt.bfloat16, tag='h1x_ld')
                    nc.sync.dma_start(
                        hx4[:], h1x_dram[M].ap()[ht0:ht0 + 4]
                        .rearrange("h p b -> p h b"))
                    for j in range(4):
                        ht = ht0 + j
                        ps = psD.tile([128, B2], dt.float32, tag='ps_h12')
                        for k in range(KT_E):
                            nc.tensor.matmul(ps[:], wblk4[:, j, k, :],
                                             inT[:, KT_E + k, :],
                                             start=(k == 0), stop=(k == KT_E - 1))
                        hpre = wp.tile([128, B2], dt.float32, tag='h1pre')
                        nc.vector.tensor_tensor(out=hpre[:], in0=ps[:],
                                                in1=hx4[:, j, :],
                                                op=mybir.AluOpType.add)
                        nc.vector.tensor_scalar_max(h1T[:, ht, :], hpre[:], 0.0)

                for ht in range(NT_HID):
                    wblk = w2p.tile([128, NT_HID, 128], dt.bfloat16, tag='w2blk')
                    lane = nc.gpsimd if ht % 2 == 0 else nc.scalar
                    lane.dma_start(wblk[:], inp[f'w2T_{M}'].ap()[ht])
                    ps = psD.tile([128, B2], dt.float32, tag='ps_h12')
                    for k in range(NT_HID):
                        nc.tensor.matmul(ps[:], wblk[:, k, :], h1T[:, k, :],
                                         start=(k == 0), stop=(k == NT_HID - 1))
                    nc.vector.tensor_scalar(
                        h2T[:, ht, :], ps[:], b2[:, ht:ht + 1], 0.0,
                        op0=mybir.AluOpType.add, op1=mybir.AluOpType.max)

                # Wc with BIT on partitions: stationary wc blocks, moving
                # h2T; bias as per-partition scalar; PE-transpose back
                wc = dp_.tile([128, NT_HID, BIT], dt.bfloat16, tag='wc')
                nc.gpsimd.dma_start(wc[:], inp[f'wcT_{M}'].ap())
                psw = psW.tile([BIT, B2], dt.float32, tag='ps_wc')
                for k in range(NT_HID):
                    nc.tensor.matmul(psw[:], wc[:, k, :], h2T[:, k, :],
                                     start=(k == 0), stop=(k == NT_HID - 1))
                h3f = spD.tile([BIT, B2], dt.float32, tag='h3f')
                nc.vector.tensor_scalar_add(h3f[:], psw[:], bcT[:, 0:1])
                for bci in range(B2 // 128):
                    pst = psT.tile([128, BIT], dt.float32, tag='ps_t')
                    nc.tensor.transpose(
                        pst[:], h3f[:, bci * 128:(bci + 1) * 128], identT[:])
                    sq = spD.tile([128, BIT], dt.float32, tag='sq')
                    ss = spD.tile([128, 1], dt.float32, tag='ss')
                    nc.scalar.activation(sq[:], pst[:], AF.Square,
                                         accum_out=ss[:])
                    rs = spD.tile([128, 1], dt.float32, tag='rs')
                    nc.vector.reciprocal(rs[:], ss[:])
                    rsq = spD.tile([128, 1], dt.float32, tag='rsq')
                    nc.scalar.sqrt(rsq[:], rs[:])
                    h3 = spD.tile([128, BIT], dt.float32, tag='h3')
                    nc.vector.tensor_scalar_mul(h3[:], pst[:], rsq[:])
                    oname = out_map[M][bci // LT]
                    row = (bci % LT) * 128
                    nc.sync.dma_start(outs[oname].ap()[row:row + 128, :], h3[:])

    nc.compile()
    return nc


def _prep_in_maps(cfg, n_cores, image_feature, text_feature, prompts,
                  img_in_w, img_in_b, img_out_w, img_out_b,
                  txt_in_w, txt_in_b, txt_out_w, txt_out_b,
                  img_W1, img_b1, img_W2, img_b2, img_Wc, img_bc,
                  txt_W1, txt_b1, txt_W2, txt_b2, txt_Wc, txt_bc):
    C = cfg
    E, P, BIT, BS = C['E'], C['P'], C['BIT'], C['BS']
    NT_HID, KT_E2 = C['NT_HID'], C['KT_E2']

    def bt(x):
        return np.ascontiguousarray(np.asarray(x).astype(BF16))

    common = {}
    common['promptsT'] = np.ascontiguousarray(prompts.T.astype(np.float32))
    pp_ = np.zeros((P, C['EPAD']), dtype=BF16)
    pp_[:, :E] = np.asarray(prompts).astype(BF16)
    pp_[:, E] = BF16(1.0)
    common['prompts_pad'] = pp_
    common['ident64'] = np.eye(BIT, dtype=np.float32)

    for m, in_w, in_b, out_w, out_b in [
            ('i', img_in_w, img_in_b, img_out_w, img_out_b),
            ('t', txt_in_w, txt_in_b, txt_out_w, txt_out_b)]:
        common[f'wqT_{m}'] = bt(in_w[:E].T)
        common[f'wkT_{m}'] = bt(in_w[E:2 * E].T)
        common[f'wvT_{m}'] = bt(in_w[2 * E:].T)
        common[f'woT_{m}'] = bt(out_w.T)
        common[f'bq_{m}'] = np.ascontiguousarray(in_b[:E].astype(np.float32))
        common[f'bk_{m}'] = bt(in_b[E:2 * E][None, :])
        common[f'bv_{m}'] = bt(in_b[2 * E:][None, :])
        common[f'bo_{m}'] = np.ascontiguousarray(out_b.astype(np.float32))

    for M, W1, b1, W2, b2, Wc, bc in [
            ('img', img_W1, img_b1, img_W2, img_b2, img_Wc, img_bc),
            ('txt', txt_W1, txt_b1, txt_W2, txt_b2, txt_Wc, txt_bc)]:
        w1t = np.asarray(W1).T.astype(BF16)      # [2E, HID]
        common[f'w1T_{M}'] = np.ascontiguousarray(
            w1t.reshape(KT_E2, 128, NT_HID, 128).transpose(2, 1, 0, 3))
        w2t = np.asarray(W2).T.astype(BF16)      # [HID, HID]
        common[f'w2T_{M}'] = np.ascontiguousarray(
            w2t.reshape(NT_HID, 128, NT_HID, 128).transpose(2, 1, 0, 3))
        wct = np.asarray(Wc).T.astype(BF16)      # [HID, BIT]
        common[f'wcT_{M}'] = np.ascontiguousarray(
            wct.reshape(NT_HID, 128, BIT).transpose(1, 0, 2))
        common[f'b1_{M}'] = np.ascontiguousarray(b1.astype(np.float32))
        common[f'b2_{M}'] = np.ascontiguousarray(b2.astype(np.float32))
        common[f'bcT_{M}'] = np.ascontiguousarray(np.asarray(bc).astype(np.float32))

    xTi = np.asarray(image_feature).T.astype(np.float32)
    xTt = np.asarray(text_feature).T.astype(np.float32)
    in_maps = []
    for c in range(n_cores):
        im = dict(common)
        im['xT_i'] = np.ascontiguousarray(xTi[:, c * BS:(c + 1) * BS])
        im['xT_t'] = np.ascontiguousarray(xTt[:, c * BS:(c + 1) * BS])
        in_maps.append(im)
    return in_maps


_NC_CACHE = {}


def _get_nc(cfg, n_cores):
    key = (tuple(sorted(cfg.items())), n_cores)
    if key not in _NC_CACHE:
        _NC_CACHE[key] = build_nc(cfg, n_cores)
    return _NC_CACHE[key]


def run(inputs, cfg=None, n_cores=None, trace=False):
    cfg = cfg or _cfg(**FULL)
    n_cores = n_cores or cfg['NC']
    nc = _get_nc(cfg, n_cores)
    in_maps = _prep_in_maps(cfg, n_cores, **{
        k: np.asarray(v) for k, v in inputs.items() if k != 'iteration'})
    res = run_bass_kernel_spmd(nc, in_maps, list(range(n_cores)), trace=trace)
    out = {}
    for name in ['image_hash', 'text_hash', 'distill_i', 'distill_t']:
        out[name] = np.concatenate(
            [res.results[c][name] for c in range(n_cores)], axis=0)
    return (out['image_hash'], out['text_hash'],
            out['distill_i'], out['distill_t']), res


def kernel(**inputs):
    (ih, th, di, dtl), _ = run(inputs)
    return ih, th, di, dtl


# revision 22
# speedup vs baseline: 1.0359x; 1.0359x over previous
"""Trainium2 Bass kernel for nn_HashingModel (retrieval_knn).

Sharding: data-parallel over batch B across 8 cores (256 rows each).
Cross-batch MHA handled by AllGather of the per-shard K/V projections
(split into separate kp and vp collectives so phase C can start on kp).
All heavy matmuls in bf16; similarity in fp32r (FP22 on the PE - 2x the
fp32 rate, zero argmax flips verified offline against fp32).
Activations flow transposed ([feature, batch] layouts) so weights can be
used pre-transposed (host-side layout prep) without on-device transposes.
Softmax skips max-subtraction (scores are tiny: weights scaled 0.02); the
denominator comes free from a ones-column appended to V.

Orchestration: three DMA lanes to avoid head-of-line blocking - sync
carries the latency-critical chain (x, prompts, idx, kp/vp shards, kpT),
scalar carries phase-A/B weight prefetch + h1x streams, gpsimd carries
vaug + phase-D weight streams (safe: its phase-C work precedes them in
queue order). PE program order: sim(i), sim(t), kvproj(i), qp(i), qp(t),
h1x(img), kvproj(t), mha(i), mha(t), h1x(txt), MLPs. The h1x precompute
(W1 @ x-half) fills the PE while the AllGathers are in flight.
Final Wc stage keeps BIT on partitions (weights stationary) and
transposes the [BIT, batch] result back via the PE with an identity.

Self-contained: hardcoded shapes, no file reads.
"""
import sys
import numpy as np

sys.path.insert(0, '/opt/trn_rl_repo')

import ml_dtypes
from concourse import bass, bacc, tile, mybir
from concourse.bass_utils import run_bass_kernel_spmd

dt = mybir.dt
BF16 = ml_dtypes.bfloat16
AF = mybir.ActivationFunctionType
F32R = dt.float32r

FULL = dict(NC=8, B=2048, E=512, P=4096, H=8, HD=64, HID=4096, BIT=64)


def _cfg(NC, B, E, P, H, HD, HID, BIT):
    c = dict(NC=NC, B=B, E=E, P=P, H=H, HD=HD, HID=HID, BIT=BIT)
    c['BS'] = B // NC          # batch shard per core
    c['E2'] = 2 * E            # MLP input dim
    c['EPAD'] = ((E + 1 + 127) // 128) * 128  # padded prompt row (ones col at E)
    c['KT_E'] = E // 128       # k-tiles over E
    c['KT_E2'] = 2 * E // 128
    c['NT_HID'] = HID // 128
    c['LT'] = c['BS'] // 128   # l-tiles per shard
    c['ST'] = B // 128         # s-tiles over full batch
    c['PC'] = P // 512         # prompt chunks for sim
    c['B2'] = 2 * c['BS']      # MLP free dim (fi|ft)
    return c


def build_nc(cfg, n_cores):
    C = cfg
    NC = n_cores
    E, P, H, HD, HID, BIT = C['E'], C['P'], C['H'], C['HD'], C['HID'], C['BIT']
    BS, E2, EPAD = C['BS'], C['E2'], C['EPAD']
    KT_E, KT_E2, NT_HID, LT, ST, PC, B2 = (C['KT_E'], C['KT_E2'], C['NT_HID'],
                                           C['LT'], C['ST'], C['PC'], C['B2'])
    HPT = 128 // HD            # heads per 128-partition tile (2)
    NHT = E // 128             # eo tiles (4)
    SEG = HD + 1               # vaug segment width (65)
    S = C['ST'] * 128          # full batch (attention keys)

    nc = bacc.Bacc("TRN2", target_bir_lowering=False, debug=False,
                   num_devices=NC)

    mods = ['i', 't']
    inp = {}

    def din(name, shape, d):
        inp[name] = nc.dram_tensor(name, shape, d, kind="ExternalInput")

    for m in mods:
        din(f'xT_{m}', [E, BS], dt.float32)
        for w in ['wqT', 'wkT', 'wvT', 'woT']:
            din(f'{w}_{m}', [E, E], dt.bfloat16)
        din(f'bq_{m}', [E], dt.float32)
        din(f'bo_{m}', [E], dt.float32)
        din(f'bk_{m}', [1, E], dt.bfloat16)
        din(f'bv_{m}', [1, E], dt.bfloat16)
    din('promptsT', [E, P], dt.float32)
    din('prompts_pad', [P, EPAD], dt.bfloat16)
    din('ident64', [BIT, BIT], dt.float32)
    for M in ['img', 'txt']:
        din(f'w1T_{M}', [NT_HID, 128, KT_E2, 128], dt.bfloat16)
        din(f'w2T_{M}', [NT_HID, 128, NT_HID, 128], dt.bfloat16)
        din(f'wcT_{M}', [128, NT_HID, BIT], dt.bfloat16)
        din(f'b1_{M}', [HID], dt.float32)
        din(f'b2_{M}', [HID], dt.float32)
        din(f'bcT_{M}', [BIT], dt.float32)

    outs = {}
    for name in ['image_hash', 'text_hash', 'distill_i', 'distill_t']:
        outs[name] = nc.dram_tensor(name, [BS, BIT], dt.float32,
                                    kind="ExternalOutput")

    idx_scr = {m: nc.dram_tensor(f'idx_scr_{m}', [BS], dt.uint32) for m in mods}
    h1x_dram = {M: nc.dram_tensor(f'h1x_{M}', [NT_HID, 128, B2], dt.bfloat16)
                for M in ['img', 'txt']}
    kp_len = E * BS
    vp_len = BS * E
    kp_in = {m: nc.dram_tensor(f'kp_in_{m}', [kp_len], dt.bfloat16) for m in mods}
    kp_out = {m: nc.dram_tensor(f'kp_out_{m}', [NC * kp_len], dt.bfloat16,
                                addr_space="Shared") for m in mods}
    vp_in = {m: nc.dram_tensor(f'vp_in_{m}', [vp_len], dt.bfloat16) for m in mods}
    vp_out = {m: nc.dram_tensor(f'vp_out_{m}', [NC * vp_len], dt.bfloat16,
                                addr_space="Shared") for m in mods}

    with tile.TileContext(nc) as tc:
      with tc.tile_pool(name="persist", bufs=1) as pp:
        xTbf = {m: pp.tile([128, KT_E, BS], dt.bfloat16, tag=f'xTbf{m}', name=f'xTbf{m}')
                for m in mods}
        inT = pp.tile([128, KT_E2, B2], dt.bfloat16, tag='inT')
        rmT = {m: pp.tile([128, EPAD // 128, BS], dt.bfloat16, tag=f'rmT{m}', name=f'rmT{m}')
               for m in mods}
        wq = {m: pp.tile([128, KT_E, E], dt.bfloat16, tag=f'wq{m}', name=f'wq{m}')
              for m in mods}
        wo = {m: pp.tile([128, KT_E, E], dt.bfloat16, tag=f'wo{m}', name=f'wo{m}')
              for m in mods}
        bq = {m: pp.tile([128, NHT], dt.float32, tag=f'bq{m}', name=f'bq{m}')
              for m in mods}
        bo = {m: pp.tile([128, NHT], dt.float32, tag=f'bo{m}', name=f'bo{m}')
              for m in mods}
        qpT = {m: pp.tile([128, NHT, BS], dt.bfloat16, tag=f'qpT{m}', name=f'qpT{m}')
               for m in mods}

        with (
            tc.tile_pool(name="phB", bufs=1) as bp_,
            tc.tile_pool(name="smB", bufs=2) as spB,
            tc.tile_pool(name="psB1", bufs=2, space="PSUM") as psB1,
        ):
            wkv = {}
            for m in mods:
                wk = bp_.tile([128, KT_E, E], dt.bfloat16, tag=f'wk{m}', name=f'wk{m}')
                wv = bp_.tile([128, KT_E, E], dt.bfloat16, tag=f'wv{m}', name=f'wv{m}')
                bk = bp_.tile([1, E], dt.bfloat16, tag=f'bk{m}', name=f'bk{m}')
                bv = bp_.tile([1, E], dt.bfloat16, tag=f'bv{m}', name=f'bv{m}')
                wkv[m] = (wk, wv, bk, bv)

            # ---- phase A: sims + argmax + gather (big tiles scoped so the
            # pool closes before phase C's K/V tiles open) ----
            with (
                tc.tile_pool(name="phSim", bufs=1) as ap_,
                tc.tile_pool(name="simbuf", bufs=3) as simp,
                tc.tile_pool(name="smA", bufs=2) as sp,
                tc.tile_pool(name="psA", bufs=3, space="PSUM") as psA,
            ):
                # sync lane: x then prompt chunks (sim critical path)
                xT32 = {m: ap_.tile([128, KT_E, BS], F32R, tag=f'xT32{m}', name=f'xT32{m}')
                        for m in mods}
                for m in mods:
                    xsrc = inp[f'xT_{m}'].ap().bitcast(F32R).rearrange(
                        "(g k p) b -> g p k b", g=2, p=128)
                    nc.sync.dma_start(xT32[m][:, 0:KT_E // 2, :], xsrc[0])
                    nc.scalar.dma_start(xT32[m][:, KT_E // 2:KT_E, :], xsrc[1])
                prc = []
                prsrc = inp['promptsT'].ap().bitcast(F32R).rearrange("(k p) n -> k p n", p=128)
                lanes = [nc.sync, nc.scalar, nc.gpsimd]
                prsrc2 = inp['promptsT'].ap().bitcast(F32R).rearrange(
                    "(g k p) n -> g p k n", g=KT_E // 2, p=128)
                for pc in range(PC):
                    t = ap_.tile([128, KT_E, 512], F32R, tag=f'prc{pc}',
                                 name=f'prc{pc}')
                    if pc == 0:
                        for k in range(KT_E):
                            nc.sync.dma_start(
                                t[:, k, :], prsrc[k][:, 0:512])
                    else:
                        for g in range(KT_E // 2):
                            lane = lanes[(pc * (KT_E // 2) + g) % 3]
                            lane.dma_start(
                                t[:, 2 * g:2 * (g + 1), :],
                                prsrc2[g][:, :, pc * 512:(pc + 1) * 512])
                    prc.append(t)
                # scalar lane: MHA weights + biases (needed from ~70us)
                for m in mods:
                    wk, wv, bk, bv = wkv[m]
                    nc.scalar.dma_start(
                        wk[:], inp[f'wkT_{m}'].ap().rearrange("(k p) n -> p k n", p=128))
                    nc.scalar.dma_start(
                        wv[:], inp[f'wvT_{m}'].ap().rearrange("(k p) n -> p k n", p=128))
                    nc.scalar.dma_start(bk[:], inp[f'bk_{m}'].ap())
                    nc.scalar.dma_start(bv[:], inp[f'bv_{m}'].ap())
                    nc.scalar.dma_start(
                        wq[m][:], inp[f'wqT_{m}'].ap().rearrange("(k p) n -> p k n", p=128))
                    nc.scalar.dma_start(
                        wo[m][:], inp[f'woT_{m}'].ap().rearrange("(k p) n -> p k n", p=128))
                    nc.scalar.dma_start(
                        bq[m][:], inp[f'bq_{m}'].ap().rearrange("(t p) -> p t", p=128))
                    nc.scalar.dma_start(
                        bo[m][:], inp[f'bo_{m}'].ap().rearrange("(t p) -> p t", p=128))
                for mi, m in enumerate(mods):
                    nc.vector.tensor_copy(xTbf[m][:], xT32[m][:])
                    nc.vector.tensor_copy(inT[:, 0:KT_E, mi * BS:(mi + 1) * BS],
                                          xTbf[m][:])

                for m in mods:
                    FRL = 128 // 16
                    for lt in range(LT):
                        sim = simp.tile([128, P], dt.float32, tag='sim')
                        xs = xT32[m][:, :, lt * 128:(lt + 1) * 128]
                        for pc in range(PC):
                            ps = psA.tile([128, 512], dt.float32, tag='ps_sim')
                            for k in range(KT_E):
                                nc.tensor.matmul(
                                    ps[:], xs[:, k, :], prc[pc][:, k, :],
                                    start=(k == 0), stop=(k == KT_E - 1))
                            nc.vector.tensor_copy(
                                sim[:, pc * 512:(pc + 1) * 512], ps[:])
                        m8 = sp.tile([128, 8], dt.float32, tag='m8')
                        i8 = sp.tile([128, 8], dt.uint32, tag=f'i8_{lt}',
                                     name=f'i8_{lt}')
                        nc.vector.max(m8[:], sim[:])
                        nc.vector.max_index(i8[:], m8[:], sim[:])
                        nc.sync.dma_start(
                            idx_scr[m].ap()[lt * 128:(lt + 1) * 128], i8[:, 0:1])
                        # per-lt gather: start fetching prompts for this half
                        # of the shard while the other half's sims still run
                        i32 = sp.tile([128, FRL], dt.uint32, tag=f'i32g{lt}',
                                      name=f'i32g{lt}')
                        for a in range(2):
                            nc.sync.dma_start(
                                i32[16 * a:16 * (a + 1), :],
                                idx_scr[m].ap()[lt * 128:(lt + 1) * 128]
                                .rearrange("(f p) -> p f", p=16))
                        for b in range(1, 4):
                            nc.vector.tensor_copy(i32[32 * b:32 * (b + 1), :],
                                                  i32[0:32, :])
                        ix16 = sp.tile([128, FRL], dt.int16, tag=f'i16g{lt}',
                                       name=f'i16g{lt}')
                        nc.vector.tensor_copy(ix16[:], i32[:])
                        rmg = sp.tile([128, EPAD // 128, 128], dt.bfloat16,
                                      tag=f'rmg{lt}', name=f'rmg{lt}')
                        nc.gpsimd.dma_gather(
                            rmg[:], inp['prompts_pad'].ap(), ix16[:],
                            num_idxs=128, num_idxs_reg=128, elem_size=EPAD,
                            transpose=True)
                        nc.vector.tensor_copy(
                            rmT[m][:, :, lt * 128:(lt + 1) * 128], rmg[:])

            # ---- phases B + C ----
            with (
                tc.tile_pool(name="phC", bufs=1) as cp_,
                tc.tile_pool(name="expp", bufs=3) as ep,
                tc.tile_pool(name="smC", bufs=1) as spC,
                tc.tile_pool(name="psO", bufs=1, space="PSUM") as psO,
                tc.tile_pool(name="psS", bufs=2, space="PSUM") as psS,
            ):
                def kvproj(m):
                    wk, wv, bk, bv = wkv[m]
                    # kpT shard -> kp_in laid out [KT_E*128, BS]
                    kpsb = spB.tile([128, NHT, BS], dt.bfloat16, tag='kpsb')
                    for eo in range(NHT):
                        psb = psB1.tile([128, 512], dt.float32, tag='ps_b',
                                        name='ps_bk')
                        ps = psb[:, 0:BS]
                        for k in range(KT_E):
                            nc.tensor.matmul(
                                ps, wk[:, k, eo * 128:(eo + 1) * 128],
                                rmT[m][:, k, :], start=(k == 0), stop=False)
                        nc.tensor.matmul(
                            ps, bk[0:1, eo * 128:(eo + 1) * 128],
                            rmT[m][0:1, KT_E, :], start=False, stop=True)
                        nc.vector.tensor_copy(kpsb[:, eo, :], ps)
                    nc.sync.dma_start(
                        kp_in[m].ap().rearrange("(q p b) -> p q b", p=128, b=BS),
                        kpsb[:])
                    nc.gpsimd.collective_compute(
                        "AllGather", mybir.AluOpType.bypass,
                        replica_groups=[list(range(NC))],
                        ins=[kp_in[m][:]], outs=[kp_out[m][:]])
                    # vp shard -> vp_in laid out [LT*128, E]
                    vpsb = spB.tile([128, LT, E], dt.bfloat16, tag='vpsb')
                    for st in range(LT):
                        ps = psB1.tile([128, E], dt.float32, tag='ps_b',
                                       name='ps_bv')
                        for k in range(KT_E):
                            nc.tensor.matmul(
                                ps[:], rmT[m][:, k, st * 128:(st + 1) * 128],
                                wv[:, k, :], start=(k == 0), stop=False)
                        nc.tensor.matmul(
                            ps[:], rmT[m][0:1, KT_E, st * 128:(st + 1) * 128],
                            bv[:], start=False, stop=True)
                        nc.vector.tensor_copy(vpsb[:, st, :], ps[:])
                    nc.sync.dma_start(
                        vp_in[m].ap().rearrange("(f p b) -> p f b", p=128, b=E),
                        vpsb[:])
                    nc.gpsimd.collective_compute(
                        "AllGather", mybir.AluOpType.bypass,
                        replica_groups=[list(range(NC))],
                        ins=[vp_in[m][:]], outs=[vp_out[m][:]])

                kpT = {}
                vaug = {}

                def load_kpT(m):
                    kpT[m] = cp_.tile([128, NHT, S], dt.bfloat16,
                                      tag=f'kpT{m}', name=f'kpT{m}')
                    ksrc = kp_out[m].ap().rearrange(
                        "(r q p b) -> q p r b", r=NC, q=NHT, p=128)
                    for eo in range(NHT):
                        lane = nc.sync if eo % 2 == 0 else nc.scalar
                        lane.dma_start(
                            kpT[m][:, eo, :].rearrange("p (r b) -> p r b", r=NC),
                            ksrc[eo])

                def load_vaug(m):
                    vaug[m] = cp_.tile([128, ST, H * SEG], dt.bfloat16,
                                       tag=f'vaug{m}', name=f'vaug{m}')
                    nc.vector.memset(
                        vaug[m][:].rearrange("p s (h d) -> p s h d", h=H)
                        [:, :, :, HD:HD + 1], 1.0)
                    vsrc = vp_out[m].ap().rearrange("(r x) -> r x", r=NC)
                    for st in range(ST):
                        r, hf = st // LT, st % LT
                        blk = vsrc[r][hf * 128 * E:(hf + 1) * 128 * E] \
                            .rearrange("(p h d) -> p h d", p=128, h=H)
                        nc.gpsimd.dma_start(
                            vaug[m][:, st, :].rearrange("p (h s) -> p h s", h=H)
                            [:, :, 0:HD], blk)

                def qproj(m):
                    for eo in range(NHT):
                        psb = psB1.tile([128, 512], dt.float32, tag='ps_b',
                                        name='ps_q')
                        ps = psb[:, 0:BS]
                        for k in range(KT_E):
                            nc.tensor.matmul(
                                ps, wq[m][:, k, eo * 128:(eo + 1) * 128],
                                xTbf[m][:, k, :], start=(k == 0),
                                stop=(k == KT_E - 1))
                        nc.vector.tensor_scalar_add(qpT[m][:, eo, :], ps,
                                                    bq[m][:, eo:eo + 1])

                def h1x(M):
                    # h1x = W1[:, x-half] @ [xT_i | xT_t] + b1 -> DRAM
                    # streams over the scalar DMA lane, 4-ht groups
                    b1x = spB.tile([128, NT_HID], dt.float32, tag='b1x')
                    nc.scalar.dma_start(
                        b1x[:], inp[f'b1_{M}'].ap().rearrange("(t p) -> p t", p=128))
                    for ht0 in range(0, NT_HID, 4):
                        wblk4 = spB.tile([128, 4, KT_E, 128], dt.bfloat16,
                                         tag='w1xblk')
                        nc.scalar.dma_start(
                            wblk4[:],
                            inp[f'w1T_{M}'].ap()[ht0:ht0 + 4][:, :, 0:KT_E]
                            .rearrange("h p k c -> p h k c"))
                        hx4 = spB.tile([128, 4, B2], dt.bfloat16, tag='h1x_sb')
                        for j in range(4):
                            ps = psB1.tile([128, B2], dt.float32, tag='ps_b',
                                           name='ps_h1x')
                            for k in range(KT_E):
                                nc.tensor.matmul(ps[:], wblk4[:, j, k, :],
                                                 inT[:, k, :],
                                                 start=(k == 0),
                                                 stop=(k == KT_E - 1))
                            nc.vector.tensor_scalar_add(
                                hx4[:, j, :], ps[:], b1x[:, ht0 + j:ht0 + j + 1])
                        nc.scalar.dma_start(
                            h1x_dram[M].ap()[ht0:ht0 + 4]
                            .rearrange("h p b -> p h b"), hx4[:])

                def mha(m, mi):
                    pso = [psO.tile([SEG, HPT * BS], dt.float32, tag=f'pso{g}',
                                    name=f'pso{g}') for g in range(H // HPT)]
                    for st2 in range(0, ST, 2):
                        ex = ep.tile([128, H, 2 * BS], dt.bfloat16, tag='expT')
                        for g in range(H // HPT):
                            for hh in range(HPT):
                                h = g * HPT + hh
                                hb = hh * HD
                                # two s-tiles share one psum bank: same PE
                                # row-group -> sequential drains, one exp op
                                pss = psS.tile([128, 2 * BS], dt.float32,
                                               tag='ps_s')
                                for sj in range(2):
                                    st = st2 + sj
                                    nc.tensor.matmul(
                                        pss[:, sj * BS:(sj + 1) * BS],
                                        kpT[m][hb:hb + HD, g,
                                               st * 128:(st + 1) * 128],
                                        qpT[m][hb:hb + HD, g, :],
                                        start=True, stop=True,
                                        skip_group_check=True)
                                nc.scalar.activation(
                                    ex[:, h, :], pss[:], AF.Exp,
                                    bias=0.0, scale=float(1.0 / np.sqrt(HD)))
                                for sj in range(2):
                                    st = st2 + sj
                                    nc.tensor.matmul(
                                        pso[g][:, hh * BS:(hh + 1) * BS],
                                        vaug[m][:, st, h * SEG:(h + 1) * SEG],
                                        ex[:, h, sj * BS:(sj + 1) * BS],
                                        start=(st == 0), stop=(st == ST - 1),
                                        skip_group_check=True)
                    zr = spC.tile([1, H * BS], dt.float32, tag='zr')
                    for h in range(H):
                        nc.vector.reciprocal(
                            zr[0:1, h * BS:(h + 1) * BS],
                            pso[h // HPT][HD:HD + 1, (h % HPT) * BS:(h % HPT + 1) * BS])
                    zb = spC.tile([HD, H * BS], dt.float32, tag='zb')
                    nc.gpsimd.partition_broadcast(zb[:], zr[:])
                    aoT = cp_.tile([128, NHT, BS], dt.bfloat16, tag='aoT')
                    for h in range(H):
                        nc.vector.tensor_tensor(
                            out=aoT[(h % HPT) * HD:(h % HPT + 1) * HD, h // HPT, :],
                            in0=pso[h // HPT][0:HD, (h % HPT) * BS:(h % HPT + 1) * BS],
                            in1=zb[:, h * BS:(h + 1) * BS],
                            op=mybir.AluOpType.mult)

                    # enhT -> inT rows E..2E-1; x -> rows 0..E-1
                    for eo in range(NHT):
                        psb = psB1.tile([128, 512], dt.float32, tag='ps_b',
                                        name='ps_e')
                        ps = psb[:, 0:BS]
                        for k in range(KT_E):
                            nc.tensor.matmul(
                                ps, wo[m][:, k, eo * 128:(eo + 1) * 128],
                                aoT[:, k, :], start=(k == 0), stop=(k == KT_E - 1))
                        nc.vector.tensor_scalar_add(
                            inT[:, KT_E + eo, mi * BS:(mi + 1) * BS], ps,
                            bo[m][:, eo:eo + 1])

                kvproj('i')
                qproj('i')
                qproj('t')
                h1x('img')
                kvproj('t')
                h1x('txt')
                load_kpT('i')
                load_vaug('i')
                load_kpT('t')
                load_vaug('t')
                mha('i', 0)
                mha('t', 1)

        # ======== Phase D: the four MLPs (two weight passes) ========
        with (
            tc.tile_pool(name="phD", bufs=1) as dp_,
            tc.tile_pool(name="w1s", bufs=2) as wp,
            tc.tile_pool(name="w2s", bufs=3) as w2p,
            tc.tile_pool(name="smD", bufs=2) as spD,
            tc.tile_pool(name="psD", bufs=4, space="PSUM") as psD,
            tc.tile_pool(name="psW", bufs=1, space="PSUM") as psW,
            tc.tile_pool(name="psT", bufs=2, space="PSUM") as psT,
        ):
            h1T = dp_.tile([128, NT_HID, B2], dt.bfloat16, tag='h1T')
            h2T = dp_.tile([128, NT_HID, B2], dt.bfloat16, tag='h2T')
            identT = dp_.tile([BIT, BIT], dt.float32, tag='ident')
            nc.sync.dma_start(identT[:], inp['ident64'].ap())
            out_map = {'img': ['image_hash', 'distill_i'],
                       'txt': ['distill_t', 'text_hash']}
            for M in ['img', 'txt']:
                b2 = spD.tile([128, NT_HID], dt.float32, tag='b2')
                bcT = spD.tile([BIT, 1], dt.float32, tag='bcT')
                nc.sync.dma_start(
                    b2[:], inp[f'b2_{M}'].ap().rearrange("(t p) -> p t", p=128))
                nc.sync.dma_start(
                    bcT[:], inp[f'bcT_{M}'].ap().rearrange("(p o) -> p o", p=BIT))

                for ht0 in range(0, NT_HID, 4):
                    wblk4 = wp.tile([128, 4, KT_E, 128], dt.bfloat16,
                                    tag='w1blk')
                    lane = nc.gpsimd if (ht0 // 4) % 2 == 0 else nc.scalar
                    lane.dma_start(
                        wblk4[:],
                        inp[f'w1T_{M}'].ap()[ht0:ht0 + 4][:, :, KT_E:KT_E2]
                        .rearrange("h p k c -> p h k c"))
                    hx4 = wp.tile([128, 4, B2], dt.bfloat16, tag='h1x_ld')
                    nc.sync.dma_start(
                        hx4[:], h1x_dram[M].ap()[ht0:ht0 + 4]
                        .rearrange("h p b -> p h b"))
                    for j in range(4):
                        ht = ht0 + j
                        ps = psD.tile([128, B2], dt.float32, tag='ps_h12')
                        for k in range(KT_E):
                            nc.tensor.matmul(ps[:], wblk4[:, j, k, :],
                                             inT[:, KT_E + k, :],
                                             start=(k == 0), stop=(k == KT_E - 1))
                        hpre = wp.tile([128, B2], dt.float32, tag='h1pre')
                        nc.vector.tensor_tensor(out=hpre[:], in0=ps[:],
                                                in1=hx4[:, j, :],
                                                op=mybir.AluOpType.add)
                        nc.vector.tensor_scalar_max(h1T[:, ht, :], hpre[:], 0.0)

                for ht in range(NT_HID):
                    wblk = w2p.tile([128, NT_HID, 128], dt.bfloat16, tag='w2blk')
                    lane = nc.gpsimd if ht % 2 == 0 else nc.scalar
                    lane.dma_start(wblk[:], inp[f'w2T_{M}'].ap()[ht])
                    ps = psD.tile([128, B2], dt.float32, tag='ps_h12')
                    for k in range(NT_HID):
                        nc.tensor.matmul(ps[:], wblk[:, k, :], h1T[:, k, :],
                                         start=(k == 0), stop=(k == NT_HID - 1))
                    nc.vector.tensor_scalar(
                        h2T[:, ht, :], ps[:], b2[:, ht:ht + 1], 0.0,
                        op0=mybir.AluOpType.add, op1=mybir.AluOpType.max)

                # Wc with BIT on partitions: stationary wc blocks, moving
                # h2T; bias as per-partition scalar; PE-transpose back
                wc = dp_.tile([128, NT_HID, BIT], dt.bfloat16, tag='wc')
                nc.gpsimd.dma_start(wc[:], inp[f'wcT_{M}'].ap())
                psw = psW.tile([BIT, B2], dt.float32, tag='ps_wc')
                for k in range(NT_HID):
                    nc.tensor.matmul(psw[:], wc[:, k, :], h2T[:, k, :],
                                     start=(k == 0), stop=(k == NT_HID - 1))
                h3f = spD.tile([BIT, B2], dt.float32, tag='h3f')
                nc.vector.tensor_scalar_add(h3f[:], psw[:], bcT[:, 0:1])
                for bci in range(B2 // 128):
                    pst = psT.tile([128, BIT], dt.float32, tag='ps_t')
                    nc.tensor.transpose(
                        pst[:], h3f[:, bci * 128:(bci + 1) * 128], identT[:])
                    sq = spD.tile([128, BIT], dt.float32, tag='sq')
                    ss = spD.tile([128, 1], dt.float32, tag='ss')
                    nc.scalar.activation(sq[:], pst[:], AF.Square,
                                         accum_out=ss[:])
                    rs = spD.tile([128, 1], dt.float32, tag='rs')
                    nc.vector.reciprocal(rs[:], ss[:])
                    rsq = spD.tile([128, 1], dt.float32, tag='rsq')
                    nc.scalar.sqrt(rsq[:], rs[:])
                    h3 = spD.tile([128, BIT], dt.float32, tag='h3')
                    nc.vector.tensor_scalar_mul(h3[:], pst[:], rsq[:])
                    oname = out_map[M][bci // LT]
                    row = (bci % LT) * 128
                    nc.sync.dma_start(outs[oname].ap()[row:row + 128, :], h3[:])

    nc.compile()
    return nc


def _prep_in_maps(cfg, n_cores, image_feature, text_feature, prompts,
                  img_in_w, img_in_b, img_out_w, img_out_b,
                  txt_in_w, txt_in_b, txt_out_w, txt_out_b,
                  img_W1, img_b1, img_W2, img_b2, img_Wc, img_bc,
                  txt_W1, txt_b1, txt_W2, txt_b2, txt_Wc, txt_bc):
    C = cfg
    E, P, BIT, BS = C['E'], C['P'], C['BIT'], C['BS']
    NT_HID, KT_E2 = C['NT_HID'], C['KT_E2']

    def bt(x):
        return np.ascontiguousarray(np.asarray(x).astype(BF16))

    common = {}
    common['promptsT'] = np.ascontiguousarray(prompts.T.astype(np.float32))
    pp_ = np.zeros((P, C['EPAD']), dtype=BF16)
    pp_[:, :E] = np.asarray(prompts).astype(BF16)
    pp_[:, E] = BF16(1.0)
    common['prompts_pad'] = pp_
    common['ident64'] = np.eye(BIT, dtype=np.float32)

    for m, in_w, in_b, out_w, out_b in [
            ('i', img_in_w, img_in_b, img_out_w, img_out_b),
            ('t', txt_in_w, txt_in_b, txt_out_w, txt_out_b)]:
        common[f'wqT_{m}'] = bt(in_w[:E].T)
        common[f'wkT_{m}'] = bt(in_w[E:2 * E].T)
        common[f'wvT_{m}'] = bt(in_w[2 * E:].T)
        common[f'woT_{m}'] = bt(out_w.T)
        common[f'bq_{m}'] = np.ascontiguousarray(in_b[:E].astype(np.float32))
        common[f'bk_{m}'] = bt(in_b[E:2 * E][None, :])
        common[f'bv_{m}'] = bt(in_b[2 * E:][None, :])
        common[f'bo_{m}'] = np.ascontiguousarray(out_b.astype(np.float32))

    for M, W1, b1, W2, b2, Wc, bc in [
            ('img', img_W1, img_b1, img_W2, img_b2, img_Wc, img_bc),
            ('txt', txt_W1, txt_b1, txt_W2, txt_b2, txt_Wc, txt_bc)]:
        w1t = np.asarray(W1).T.astype(BF16)      # [2E, HID]
        common[f'w1T_{M}'] = np.ascontiguousarray(
            w1t.reshape(KT_E2, 128, NT_HID, 128).transpose(2, 1, 0, 3))
        w2t = np.asarray(W2).T.astype(BF16)      # [HID, HID]
        common[f'w2T_{M}'] = np.ascontiguousarray(
            w2t.reshape(NT_HID, 128, NT_HID, 128).transpose(2, 1, 0, 3))
        wct = np.asarray(Wc).T.astype(BF16)      # [HID, BIT]
        common[f'wcT_{M}'] = np.ascontiguousarray(
            wct.reshape(NT_HID, 128, BIT).transpose(1, 0, 2))
        common[f'b1_{M}'] = np.ascontiguousarray(b1.astype(np.float32))
        common[f'b2_{M}'] = np.ascontiguousarray(b2.astype(np.float32))
        common[f'bcT_{M}'] = np.ascontiguousarray(np.asarray(bc).astype(np.float32))

    xTi = np.asarray(image_feature).T.astype(np.float32)
    xTt = np.asarray(text_feature).T.astype(np.float32)
    in_maps = []
    for c in range(n_cores):
        im = dict(common)
        im['xT_i'] = np.ascontiguousarray(xTi[:, c * BS:(c + 1) * BS])
        im['xT_t'] = np.ascontiguousarray(xTt[:, c * BS:(c + 1) * BS])
        in_maps.append(im)
    return in_maps


_NC_CACHE = {}


def _get_nc(cfg, n_cores):
    key = (tuple(sorted(cfg.items())), n_cores)
    if key not in _NC_CACHE:
        _NC_CACHE[key] = build_nc(cfg, n_cores)
    return _NC_CACHE[key]


def run(inputs, cfg=None, n_cores=None, trace=False):
    cfg = cfg or _cfg(**FULL)
    n_cores = n_cores or cfg['NC']
    nc = _get_nc(cfg, n_cores)
    in_maps = _prep_in_maps(cfg, n_cores, **{
        k: np.asarray(v) for k, v in inputs.items() if k != 'iteration'})
    res = run_bass_kernel_spmd(nc, in_maps, list(range(n_cores)), trace=trace)
    out = {}
    for name in ['image_hash', 'text_hash', 'distill_i', 'distill_t']:
        out[name] = np.concatenate(
            [res.results[c][name] for c in range(n_cores)], axis=0)
    return (out['image_hash'], out['text_hash'],
            out['distill_i'], out['distill_t']), res


def kernel(**inputs):
    (ih, th, di, dtl), _ = run(inputs)
    return ih, th, di, dtl
